# revision 33
# baseline (speedup 1.0000x reference)
"""GriffinBlock1D Trainium2 Bass kernel (v3).

Sharding: 8 cores = (batch b, T-half). Each core computes GLRU only over
XT=640 tokens: [h0-112, h0+528) with a 96-token scan warmup prefix (carry
decays like prod(sigmoid) ~ e^-0.8/token, so truncation error ~e^-28) plus
the +-16 attention halo. Out-of-range tokens are zero-padded on the host
(u=0.5, cand=0 -> scan carry stays exact for half 0).

v3 changes vs v2:
- GLRU warmup-prefix instead of full-T replication: halves GLRU matmuls,
  sigmoids, and scan length; removes all cw-select windowing DVE ops.
- LN3 deleted: with attn_g=1/attn_b=0 the LN2 output already has exact
  zero mean / unit variance per token, so the FFN pre-LN is the identity
  to ~1e-5.
- w = (1-u)*cand in ONE DVE scalar_tensor_tensor with host-negated wc
  ((u-1)*(-cand)), dropping the second sigmoid activation per tile.
- All DRAM tensors stored in final SBUF layout: every dma_start moves
  [128, N]-contiguous rows (4-16KB descriptors, not 1KB).
- w1/w2 (4MB) DMA-gated behind the GLRU-critical loads via tiny copies.
- Chunked scans with carry chaining; chunked LN1 feeding chunked q/k
  projections; x3 written directly as bf16; LN squares on DVE; q/k
  PSUM->SBUF copies on the scalar engine (Copy lives in every act table).
- Activation-table prefetch dummies before each table switch.
"""

import numpy as np
import ml_dtypes

import concourse.bass as bass
import concourse.mybir as mybir
import concourse.tile as tile
from concourse import bacc
from concourse.bass_utils import run_bass_kernel_spmd

F32 = mybir.dt.float32
BF16 = mybir.dt.bfloat16
AF = mybir.ActivationFunctionType
ALU = mybir.AluOpType

B, T, D, H, WIN, FFD = 4, 1024, 512, 4, 16, 2048
DH = D // H          # 128
TL = T // 2          # 512 tokens per core
WARM = 96            # scan warmup prefix
XT = WARM + WIN + TL + WIN   # 640 tokens loaded/scanned per core
WT = TL + 2 * WIN    # 544-token attention window (with halo)
EPS = 1e-5
SCL = 1.0 / np.sqrt(DH)
NCORES = 8

# attention geometry: q-blocks in x1-column space [16, 528); k-pieces of
# <=128 rows; (q0, qn, [(pst, plen), ...], mask col offset)
QBLOCKS = [
    (16, 224, [(0, 128), (128, 128)], 0),
    (240, 224, [(224, 128), (352, 128)], 448),
    (464, 64, [(448, 96)], 896),
]
VSTARTS = [(0, 128), (128, 128), (224, 128), (352, 128), (448, 96)]
MSKW = 1152

_CACHE = {}


def _patch_act_tables():
    """Make Ln/Exp resolve to the combined natural_log_exp table so the
    scalar engine does not reload its function table between Ln and Exp."""
    if _CACHE.get("act_patched"):
        return
    from concourse.hw_specs import get_activation_tables as orig

    def patched(arch):
        t = dict(orig(arch))
        keep = "natural_log_exp_and_others"
        if keep in t:
            for name in list(t):
                if name != keep:
                    t[name] = t[name] - {AF.Ln, AF.Exp}
        return t

    bacc.get_activation_tables = patched
    _CACHE["act_patched"] = True


def _build_nc(ln_id):
    # ln_id[i]: LayerNorm i has gamma==1 and beta==0 (skip affine pass)
    _patch_act_tables()
    nc = bacc.Bacc("TRN2", target_bir_lowering=False, debug=False)

    di = lambda n, s, dt=BF16: nc.dram_tensor(n, s, dt, kind="ExternalInput")
    xt_d = di("xt", [128, 4, XT])
    wuc_d = di("wuc", [128, 8, D])         # W_in[:D].T | -(W_state@W_in[D:]).T
    wq_d = di("wq", [128, 4, D])
    wk_d = di("wk", [128, 4, D])
    wv_d = di("wv", [128, 4, D])
    wp_d = di("wp", [128, 4, D])
    w1_d = di("w1", [128, 4, FFD])
    w2_d = di("w2", [128, 16, D])
    lncol_d = di("lncol", [128, 4, 8], F32)   # [p, et, 2*ln + (g|b)]
    b1_d = di("b1c", [128, FFD // 128], F32)
    b2_d = di("b2c", [128, D // 128], F32)
    msk_d = di("msk", [128, MSKW], BF16)
    out_d = nc.dram_tensor("outp", [4, 128, TL], BF16, kind="ExternalOutput")

    skip_ln3 = ln_id[1] and ln_id[2]

    with nc.allow_low_precision(reason="bf16 activations; LN renormalizes"), \
         tile.TileContext(nc) as tc:
        # PSUM tags (8 banks total): mm(2) + cd(2) + sc(2) + s12(2)
        with tc.tile_pool(name="cp", bufs=1) as cp, \
             tc.tile_pool(name="sq", bufs=2) as sqp, \
             tc.tile_pool(name="lnt", bufs=2) as lnp, \
             tc.tile_pool(name="ep", bufs=4) as ep, \
             tc.tile_pool(name="pp", bufs=2, space="PSUM") as pm:

            # ---------------- inputs / constants ----------------
            # phase-1 loads (GLRU-critical): xt per-kk so the first matmuls
            # can issue as soon as their stripe lands; wu half before wc half
            xt_sb = cp.tile([128, 4, XT], BF16, tag="xt")
            wuc_sb = cp.tile([128, 8, D], BF16, tag="wuc")
            nc.sync.dma_start(xt_sb[:, 0, :], xt_d[:, 0, :])
            nc.sync.dma_start(wuc_sb[:, 0:4, :], wuc_d[:, 0:4, :])
            nc.sync.dma_start(xt_sb[:, 1, :], xt_d[:, 1, :])
            nc.sync.dma_start(xt_sb[:, 2, :], xt_d[:, 2, :])
            nc.sync.dma_start(xt_sb[:, 3, :], xt_d[:, 3, :])
            nc.sync.dma_start(wuc_sb[:, 4:8, :], wuc_d[:, 4:8, :])
            b1_sb = cp.tile([128, FFD // 128], F32, tag="b1")
            nc.sync.dma_start(b1_sb, b1_d[:, :])
            b2_sb = cp.tile([128, D // 128], F32, tag="b2")
            nc.sync.dma_start(b2_sb, b2_d[:, :])

            # phase-2 loads (attention): gated behind wuc arrival
            gate = lambda dst, src: nc.vector.tensor_copy(
                dst[(slice(0, 1),) + (0,) * (len(dst.shape) - 2) + (slice(0, 1),)],
                src[(slice(0, 1),) + (0,) * (len(src.shape) - 2) + (slice(0, 1),)])
            wqk_sb = cp.tile([128, 8, D], BF16, tag="wqk")
            gate(wqk_sb, wuc_sb)
            nc.sync.dma_start(wqk_sb[:, 0:4, :], wq_d[:, :, :])
            nc.sync.dma_start(wqk_sb[:, 4:8, :], wk_d[:, :, :])
            wv_sb = cp.tile([128, 4, D], BF16, tag="wv")
            gate(wv_sb, wuc_sb)
            nc.sync.dma_start(wv_sb, wv_d[:, :, :])
            wp_sb = cp.tile([128, 4, D], BF16, tag="wp")
            gate(wp_sb, wuc_sb)
            nc.sync.dma_start(wp_sb, wp_d[:, :, :])
            msk_sb = cp.tile([128, MSKW], BF16, tag="msk")
            gate(msk_sb, wuc_sb)
            nc.sync.dma_start(msk_sb, msk_d[:, :])
            lncol_sb = cp.tile([128, 4, 8], F32, tag="lncol")
            nc.sync.dma_start(lncol_sb, lncol_d[:, :, :])

            # phase-3 loads (FFN, 4MB): gated behind wp arrival
            w1_sb = cp.tile([128, 4, FFD], BF16, tag="w1")
            gate(w1_sb, wp_sb)
            nc.sync.dma_start(w1_sb, w1_d[:, :, :])
            w2_sb = cp.tile([128, 16, D], BF16, tag="w2")
            gate(w2_sb, wp_sb)
            nc.sync.dma_start(w2_sb, w2_d[:, :, :])

            ones_sb = cp.tile([128, 128], BF16, tag="ones")
            nc.vector.memset(ones_sb, 1.0)
            epsc = cp.tile([128, 1], F32, tag="epsc")
            nc.vector.memset(epsc, EPS)

            # PE warmup: dummy matmuls while the input DMAs land, so the HAM
            # clock gate is released (1.2 -> 2.4 GHz) before the real work
            for wi in range(24):
                warm = pm.tile([128, 128], F32, tag="sc")
                nc.tensor.matmul(warm, ones_sb, ones_sb, start=True, stop=True)

            # ---------------- GLRU: u / cand matmuls + chained scan ----------------
            u_sb = cp.tile([128, 4, XT], BF16, tag="u")
            w_sb = cp.tile([128, 4, XT], F32, tag="w")
            y_sb = cp.tile([128, 4, XT], BF16, tag="y")
            GCH = [(0, 512), (512, XT - 512)]
            # chunk-outer so all four ets' chunk-0 scans finish early and
            # LN1 chunk 0 / qkv can start while chunk-1 matmuls still run
            for (c0, cn) in GCH:
                for et in range(4):
                    csl = slice(c0, c0 + cn)
                    g1 = pm.tile([128, cn], F32, tag="mm")
                    for kk in range(4):
                        nc.tensor.matmul(
                            g1, wuc_sb[:, kk, et * 128:(et + 1) * 128],
                            xt_sb[:, kk, csl], start=kk == 0, stop=kk == 3)
                    nc.scalar.activation(u_sb[:, et, csl], g1, AF.Sigmoid)
                    cd = pm.tile([128, cn], F32, tag="cd")
                    for kk in range(4):
                        nc.tensor.matmul(
                            cd, wuc_sb[:, 4 + kk, et * 128:(et + 1) * 128],
                            xt_sb[:, kk, csl], start=kk == 0, stop=kk == 3)
                    # w = (u-1)*(-cand) = (1-u)*cand  (wc is host-negated)
                    nc.vector.scalar_tensor_tensor(
                        w_sb[:, et, csl], u_sb[:, et, csl], 1.0, cd,
                        ALU.subtract, ALU.mult)
                    # chained scan per chunk (carry = last col of prev chunk)
                    init = 0.0 if c0 == 0 else y_sb[:, et, c0 - 1:c0]
                    nc.vector.tensor_tensor_scan(
                        y_sb[:, et, csl], u_sb[:, et, csl], w_sb[:, et, csl],
                        init, ALU.mult, ALU.add)

            # preload the ln/exp activation table right after the last sigmoid
            tbl = sqp.tile([1, 1], BF16, tag="tbl")
            nc.scalar.activation(tbl, u_sb[0:1, 3, XT - 1:XT], AF.Ln)

            # ---------------- LayerNorm (broadcast-form stats) ----------------
            # pointwise work spread across engines: squares on sq_eng,
            # (x - m) on gpsimd (idle engine, SBUF-only operands), * r on DVE
            def layer_norm(xin, ln_idx, out_get, ncols, chunk, f32_stats=False,
                           sq_eng=None, sub_eng=None, post_et=None):
                sdt = F32 if f32_stats else BF16
                sq_eng = sq_eng or nc.gpsimd
                sub_eng = sub_eng or nc.gpsimd
                for c0 in range(0, ncols, chunk):
                    cn = min(chunk, ncols - c0)
                    cs = slice(c0, c0 + cn)
                    s1b = pm.tile([128, cn], F32, tag="s12")
                    s2b = pm.tile([128, cn], F32, tag="s12")
                    for et in range(4):
                        sq = sqp.tile([128, cn], BF16, tag="sq")
                        if sq_eng is nc.scalar:
                            nc.scalar.activation(sq, xin(et)[:, cs], AF.Square)
                        else:
                            sq_eng.tensor_mul(sq, xin(et)[:, cs], xin(et)[:, cs])
                        nc.tensor.matmul(s1b, ones_sb, xin(et)[:, cs],
                                         start=et == 0, stop=et == 3)
                        nc.tensor.matmul(s2b, ones_sb, sq,
                                         start=et == 0, stop=et == 3)
                    mb = lnp.tile([128, cn], sdt, tag="mb")
                    nc.scalar.activation(mb, s1b, AF.Copy, scale=1.0 / D)
                    m2b = lnp.tile([128, cn], sdt, tag="m2b")
                    nc.scalar.activation(m2b, s1b, AF.Square, scale=1.0 / D)
                    vb = lnp.tile([128, cn], sdt, tag="vb")
                    nc.vector.scalar_tensor_tensor(
                        vb, s2b, 1.0 / D, m2b, ALU.mult, ALU.subtract)
                    lnv = lnp.tile([128, cn], sdt, tag="lnv")
                    nc.scalar.activation(lnv, vb, AF.Ln, bias=epsc)
                    rb = lnp.tile([128, cn], sdt, tag="rb")
                    nc.scalar.activation(rb, lnv, AF.Exp, scale=-0.5)
                    for et in range(4):
                        o = out_get(et)[:, cs]
                        t1 = sqp.tile([128, cn], BF16, tag="t1")
                        sub_eng.tensor_sub(t1, xin(et)[:, cs], mb)
                        if ln_id[ln_idx]:
                            nc.vector.tensor_mul(o, t1, rb)
                        else:
                            nc.vector.tensor_mul(t1, t1, rb)
                            g_ap = lncol_sb[:, et, 2 * ln_idx:2 * ln_idx + 1]
                            b_ap = lncol_sb[:, et, 2 * ln_idx + 1:2 * ln_idx + 2]
                            nc.scalar.activation(o, t1, AF.Identity,
                                                 scale=g_ap, bias=b_ap)
                        if post_et is not None:
                            post_et(et, cs)

            # ---------------- LN1: x1 = LN(y[WARM:WARM+WT]) ----------------
            x1 = cp.tile([128, 4, WT], BF16, tag="x1")
            yw = lambda et: y_sb[:, et, WARM:WARM + WT]
            layer_norm(yw, 0, lambda et: x1[:, et, :], WT, chunk=272,
                       sq_eng=nc.scalar)

            # ---------------- attention: q/k/v projections ----------------
            # split matmuls at the LN1 chunk boundary (x1 col 272) so the
            # first halves start while LN1 chunk 1 is still normalizing
            q_sb = cp.tile([128, 4, TL], BF16, tag="q")
            k_sb = cp.tile([128, 4, WT], BF16, tag="k")
            for h in range(4):
                qp = pm.tile([128, TL], F32, tag="mm")
                for (a0, an) in [(0, 256), (256, 256)]:
                    for kk in range(4):
                        nc.tensor.matmul(
                            qp[:, a0:a0 + an],
                            wqk_sb[:, kk, h * 128:(h + 1) * 128],
                            x1[:, kk, WIN + a0:WIN + a0 + an],
                            start=kk == 0, stop=kk == 3)
                nc.vector.tensor_copy(q_sb[:, h, :], qp)
                kp = pm.tile([128, TL], F32, tag="mm")
                for (a0, an) in [(0, 272), (272, 240)]:
                    for kk in range(4):
                        nc.tensor.matmul(
                            kp[:, a0:a0 + an],
                            wqk_sb[:, 4 + kk, h * 128:(h + 1) * 128],
                            x1[:, kk, a0:a0 + an],
                            start=kk == 0, stop=kk == 3)
                kp2 = pm.tile([128, 32], F32, tag="sc")
                for kk in range(4):
                    nc.tensor.matmul(kp2,
                                     wqk_sb[:, 4 + kk, h * 128:(h + 1) * 128],
                                     x1[:, kk, TL:WT], start=kk == 0, stop=kk == 3)
                nc.scalar.activation(k_sb[:, h, 0:TL], kp, AF.Copy)
                nc.scalar.activation(k_sb[:, h, TL:WT], kp2, AF.Copy)

            # v token-major: [t' (part), d] chunks at x1-cols VSTARTS
            v_sb = cp.tile([128, 5, D], BF16, tag="v")
            for ci, (st, rows) in enumerate(VSTARTS):
                vp = pm.tile([128, D], F32, tag="cd")
                for kk in range(4):
                    nc.tensor.matmul(vp[0:rows, :],
                                     x1[:, kk, st:st + rows],
                                     wv_sb[:, kk, :],
                                     start=kk == 0, stop=kk == 3)
                nc.vector.tensor_copy(v_sb[0:rows, ci, :], vp[0:rows, :])

            # ---------------- banded softmax attention ----------------
            a2 = cp.tile([128, 4, TL], BF16, tag="a2")
            rec_sb = cp.tile([128, TL], F32, tag="rec")
            # C-block (64 q-cols, one 96-row piece) scores for all 4 heads go
            # into one PSUM bank -> single exp + mask for 4 heads
            spc = pm.tile([128, 448], F32, tag="sc")
            ec = ep.tile([128, 448], BF16, tag="e")
            q0c, qnc, (pstc, plenc) = QBLOCKS[2][0], QBLOCKS[2][1], QBLOCKS[2][2][0]
            for h in range(4):
                nc.tensor.matmul(spc[0:plenc, h * 64:h * 64 + qnc],
                                 k_sb[:, h, pstc:pstc + plenc],
                                 q_sb[:, h, q0c - WIN:q0c - WIN + qnc],
                                 start=True, stop=True)
            nc.scalar.activation(ec[:, 0:256], spc[:, 0:256], AF.Exp, scale=SCL)
            nc.vector.tensor_mul(ec[:, 0:256], ec[:, 0:256],
                                 msk_sb[:, 896:1152])
            for h in range(4):
                den = pm.tile([128, TL], F32, tag="cd")
                ao = pm.tile([128, TL], F32, tag="s12")
                for q0, qn, pieces, mcol in QBLOCKS[:2]:
                    qsl = slice(q0 - WIN, q0 - WIN + qn)   # q_sb index space
                    bsl = slice(q0 - WIN, q0 - WIN + qn)   # block cols in den/ao
                    npc = len(pieces)
                    sp = pm.tile([128, 448], F32, tag="sc")
                    for pi, (pst, plen) in enumerate(pieces):
                        nc.tensor.matmul(sp[0:plen, pi * 224:pi * 224 + qn],
                                         k_sb[:, h, pst:pst + plen],
                                         q_sb[:, h, qsl],
                                         start=True, stop=True)
                    mw = (npc - 1) * 224 + qn
                    e = ep.tile([128, 448], BF16, tag="e")
                    nc.scalar.activation(e[:, 0:mw], sp[:, 0:mw],
                                         AF.Exp, scale=SCL)
                    nc.vector.tensor_mul(e[:, 0:mw], e[:, 0:mw],
                                         msk_sb[:, mcol:mcol + mw])
                    for pi, (pst, plen) in enumerate(pieces):
                        esl = e[0:plen, pi * 224:pi * 224 + qn]
                        nc.tensor.matmul(den[:, bsl], ones_sb[0:plen, :], esl,
                                         start=pi == 0, stop=pi == npc - 1)
                        ci = VSTARTS.index((pst, plen))
                        nc.tensor.matmul(ao[:, bsl],
                                         v_sb[0:plen, ci, h * 128:(h + 1) * 128],
                                         esl,
                                         start=pi == 0, stop=pi == npc - 1)
                eslc = ec[0:plenc, h * 64:h * 64 + qnc]
                bslc = slice(q0c - WIN, q0c - WIN + qnc)
                cic = VSTARTS.index((pstc, plenc))
                nc.tensor.matmul(den[:, bslc], ones_sb[0:plenc, :], eslc,
                                 start=True, stop=True)
                nc.tensor.matmul(ao[:, bslc],
                                 v_sb[0:plenc, cic, h * 128:(h + 1) * 128],
                                 eslc, start=True, stop=True)
                # 1/den on DVE (den >= ~e^-16 > 0, no edge cases)
                nc.vector.reciprocal_approx_fast(rec_sb, den)
                nc.vector.tensor_mul(a2[:, h, :], ao, rec_sb)

            # ---------------- proj + residual, LN2 ----------------
            x2pre = cp.tile([128, 4, TL], BF16, tag="x2pre")
            for et in range(4):
                pp = pm.tile([128, TL], F32, tag="mm")
                for kk in range(4):
                    nc.tensor.matmul(pp, wp_sb[:, kk, et * 128:(et + 1) * 128],
                                     a2[:, kk, :], start=kk == 0, stop=kk == 3)
                nc.vector.tensor_add(x2pre[:, et, :],
                                     x1[:, et, WIN:WIN + TL], pp)
            x2 = cp.tile([128, 4, TL], BF16, tag="x2")
            layer_norm(lambda et: x2pre[:, et, :], 1,
                       lambda et: x2[:, et, :], TL, chunk=256,
                       sq_eng=nc.scalar, sub_eng=nc.vector)

            # ---------------- FFN pre-LN: identity when LN2/LN3 affines are
            # identity (LN2 output already has zero mean / unit variance)
            if skip_ln3:
                xf = x2
            else:
                xf = cp.tile([128, 4, TL], BF16, tag="xf")
                layer_norm(lambda et: x2[:, et, :], 2,
                           lambda et: xf[:, et, :], TL, chunk=256)

            # preload the gelu table once the last x2 chunk is written
            tbl2 = sqp.tile([1, 1], BF16, tag="tbl")
            nc.scalar.activation(tbl2, x2[0:1, 3, TL - 1:TL], AF.Gelu)

            # ---------------- FFN ----------------
            hg = cp.tile([128, 16, TL], BF16, tag="hg")
            ops = [pm.tile([128, TL], F32, tag="mm", name="op0"),
                   pm.tile([128, TL], F32, tag="mm", name="op1"),
                   pm.tile([128, TL], F32, tag="cd", name="op2"),
                   pm.tile([128, TL], F32, tag="cd", name="op3")]
            for kk in range(16):
                hp = pm.tile([128, TL], F32, tag="sc")
                for ki in range(4):
                    nc.tensor.matmul(hp, w1_sb[:, ki, kk * 128:(kk + 1) * 128],
                                     xf[:, ki, :], start=ki == 0, stop=ki == 3)
                nc.scalar.activation(hg[:, kk, :], hp, AF.Gelu,
                                     bias=b1_sb[:, kk:kk + 1])
                for et in range(4):
                    nc.tensor.matmul(ops[et], w2_sb[:, kk, et * 128:(et + 1) * 128],
                                     hg[:, kk, :],
                                     start=kk == 0, stop=kk == 15)
            # preload the ln/exp table after the last gelu
            tbl3 = sqp.tile([1, 1], BF16, tag="tbl")
            nc.scalar.activation(tbl3, hg[0:1, 15, TL - 1:TL], AF.Ln)

            x3 = cp.tile([128, 4, TL], BF16, tag="x3")
            for et in range(4):
                nc.vector.scalar_tensor_tensor(
                    x3[:, et, :], ops[et], b2_sb[:, et:et + 1],
                    x2[:, et, :], ALU.add, ALU.add)

            # ---------------- LN4 -> output ----------------
            outt = cp.tile([128, 4, TL], BF16, tag="outt")
            layer_norm(lambda et: x3[:, et, :], 3, lambda et: outt[:, et, :],
                       TL, chunk=256, f32_stats=True, sq_eng=nc.scalar,
                       sub_eng=nc.vector,
                       post_et=lambda et, cs: nc.sync.dma_start(
                           out_d[et, :, cs], outt[:, et, cs]))

    nc.compile()
    return nc


def _pack4(m):
    """[512, X] host matrix -> [128, 4, X] SBUF layout (p, a, e)."""
    x = np.ascontiguousarray(np.asarray(m).reshape(4, 128, -1).transpose(1, 0, 2))
    return x


def _host_inputs(x, W_in, W_state, glru_g, glru_b, Wq, Wk, Wv, Wp, attn_g,
                 attn_b, ffn_g, ffn_b, W1, b1, W2, b2, out_g, out_b):
    bf = ml_dtypes.bfloat16
    f32 = np.float32
    cb = lambda a: _pack4(np.asarray(a, dtype=f32)).astype(bf)
    W_in = np.asarray(W_in, f32)
    W_state = np.asarray(W_state, f32)
    wc_mat = -(W_state @ W_in[D:])           # negated: w = (u-1)*(-cand)
    # lncol[p, et, 2*ln+(g|b)] : per-feature gamma/beta columns
    lncol = np.zeros((128, 4, 8), f32)
    for ln, (g, b) in enumerate([(glru_g, glru_b), (attn_g, attn_b),
                                 (ffn_g, ffn_b), (out_g, out_b)]):
        g = np.asarray(g, f32).reshape(4, 128)
        b = np.asarray(b, f32).reshape(4, 128)
        for et in range(4):
            lncol[:, et, 2 * ln] = g[et]
            lncol[:, et, 2 * ln + 1] = b[et]
    w2p = np.ascontiguousarray(
        np.asarray(W2, f32).T.reshape(16, 128, D).transpose(1, 0, 2)).astype(bf)
    wuc = np.concatenate([cb(W_in[:D].T), cb(wc_mat.T)], axis=1)
    shared = {
        "wuc": np.ascontiguousarray(wuc),
        "wq": cb(np.asarray(Wq, f32).T), "wk": cb(np.asarray(Wk, f32).T),
        "wv": cb(np.asarray(Wv, f32).T), "wp": cb(np.asarray(Wp, f32).T),
        "w1": cb(np.asarray(W1, f32).T), "w2": w2p,
        "lncol": lncol,
        "b1c": np.ascontiguousarray(
            np.asarray(b1, f32).reshape(FFD // 128, 128).T),
        "b2c": np.ascontiguousarray(
            np.asarray(b2, f32).reshape(D // 128, 128).T),
    }
    xf32 = np.asarray(x, f32)
    in_maps = []
    for core in range(NCORES):
        b_, half = core // 2, core % 2
        h0 = half * TL
        m = dict(shared)
        # xt slice: tokens [h0 - WARM - WIN, h0 + TL + WIN), zero-padded
        t0 = h0 - WARM - WIN
        sl = np.zeros((XT, D), f32)
        lo, hi = max(t0, 0), min(t0 + XT, T)
        sl[lo - t0:hi - t0] = xf32[b_, lo:hi]
        m["xt"] = _pack4(sl.T).astype(bf)     # [128, 4, XT]
        # masks per (block, piece): band |kc-qc|<=16 and true k-token in range
        msk = np.zeros((128, MSKW), f32)
        for q0, qn, pieces, mcol in QBLOCKS:
            for pi, (pst, plen) in enumerate(pieces):
                r = np.arange(plen)
                c = np.arange(qn)
                kc = pst + r
                qc = q0 + c
                tk = h0 - WIN + kc
                band = (np.abs(kc[:, None] - qc[None, :]) <= WIN) \
                    & (tk[:, None] >= 0) & (tk[:, None] < T)
                if mcol == 896:   # C block: replicate for the 4 heads
                    for hh in range(4):
                        msk[0:plen, mcol + hh * 64:mcol + hh * 64 + qn] = band
                else:
                    msk[0:plen, mcol + pi * 224:mcol + pi * 224 + qn] = band
        m["msk"] = msk.astype(bf)
        in_maps.append(m)
    return in_maps


def kernel(**inputs):
    lnpairs = [("glru_g", "glru_b"), ("attn_g", "attn_b"),
               ("ffn_g", "ffn_b"), ("out_g", "out_b")]
    ln_id = tuple(
        bool(np.all(np.asarray(inputs[gn]) == 1.0)
             and np.all(np.asarray(inputs[bn]) == 0.0))
        for gn, bn in lnpairs)
    key = ("nc", ln_id)
    if key not in _CACHE:
        _CACHE[key] = _build_nc(ln_id)
    nc = _CACHE[key]
    in_maps = _host_inputs(**inputs)
    res = run_bass_kernel_spmd(nc, in_maps, core_ids=list(range(NCORES)),
                               **_CACHE.get("run_kwargs", {}))
    _CACHE["last_result"] = res
    out = np.empty((B, T, D), np.float32)
    for core in range(NCORES):
        b_, half = core // 2, core % 2
        o = np.asarray(res.results[core]["outp"], dtype=np.float32)  # [4,128,TL]
        out[b_, half * TL:(half + 1) * TL, :] = o.reshape(D, TL).T
    return out
